# revision 10
# baseline (speedup 1.0000x reference)
"""Trainium2 Bass kernel for the Graves-attention RNN (nn_CustomRNNCell).

Strategy: data-parallel over batch B=128 across 8 NeuronCores (16 rows/core).
Each core runs the full T=512 sequential scan in one Bass/Tile program.

Per-core layouts (b = 16 local batch rows):
  - h state kept transposed  h_T [128p, 4k, 16b]  (H=512 on partitions, 4 chunks)
  - z = inp@Wx + h@Wh computed batch-major with weights as the moving operand
    and col-tiling (tile_position) so the 4 gate banks stream concurrently:
    zb [128p, 512f] with gate g living at partitions 32g..32g+16.
  - LSTM pointwise batch-major on [16,512] slabs (merged sigmoid over i,f,o).
  - attention window computed in u-on-partitions layout [128u, (b,m)=160f]
    via exponent polynomial  s = c0 + c1*u + c2*u^2  (per-partition scalar u),
    with c0/c1/c2 spread from batch-major to 160 free cols by a masked
    broadcast matmul (ones stationary, fp32 exact).
  - w_t = wfull[:,:U] @ trans[b] via 16 small matmuls (trans[b] stationary).
  - outputs stored to DRAM in native layouts; host reassembles/transposes.
"""

import os
import sys

sys.path.insert(0, "/opt/trn_rl_repo")

import numpy as np

import concourse.bass as bass
import concourse.bacc as bacc
import concourse.mybir as mybir
import concourse.tile as tile
from concourse.bass import ds
from concourse.bass_utils import run_bass_kernel_spmd

F32 = mybir.dt.float32
F32R = mybir.dt.float32r
U32 = mybir.dt.uint32

# Problem constants (hardcoded per contest rules)
B, T_FULL, U, N, H, M = 128, 512, 128, 80, 512, 10
NCORES = 8
BL = B // NCORES  # 16 local batch rows
G4 = 4 * H  # 2048 gate dim
IN_DIM = 3 + N  # 83
IN_P = 128  # padded inp rows: w at 0:80, x at 96:99, rest zero

AF = mybir.ActivationFunctionType
ALU = mybir.AluOpType


def build_program(T, z_dtype="f32r", y_dtype="f32r"):
    """Build the SPMD Bass program for T timesteps. Returns nc."""
    nc = bacc.Bacc(None, target_bir_lowering=False)

    # ---------------- DRAM parameters (per-core) ----------------
    def din(name, shape):
        return nc.declare_dram_parameter(name, list(shape), F32, isOutput=False)

    strokes_t = din("strokes_t", [3, T * BL])          # x transposed, [3, t*16+b]
    trans_u = din("trans_u", [U, BL * N])              # [u, b*80+n]
    wx = din("wx", [IN_P, G4])                         # [128, 2048] padded
    wh = din("wh", [H, G4])                            # [512, 2048]
    wd = din("wd", [H, 3 * M])                         # [512, 30]
    bd2 = din("bd2", [BL, 3 * M])                      # bd broadcast [16, 30]
    masks = din("masks", [BL, 3 * BL * M])             # [16, 480] c0/c1/c2 spread masks
    ones16 = din("ones16", [BL, 128])                  # ones [16,128]
    id16 = din("id16", [BL, BL])                       # eye(16)
    id128 = din("id128", [128, 128])                   # eye(128)
    enums = din("enums", [128, 2])                     # [u, u^2] per partition

    o_h = nc.declare_dram_parameter("o_h", [T, BL, H], F32, isOutput=True)
    o_w = nc.declare_dram_parameter("o_w", [T, N, BL], F32, isOutput=True)
    o_idx = nc.declare_dram_parameter("o_idx", [T, BL], U32, isOutput=True)

    zdt = F32R if z_dtype == "f32r" else F32
    ydt = F32R if y_dtype == "f32r" else F32

    def bc(ap, dt_):
        return ap.bitcast(dt_) if dt_ is not F32 else ap

    with tile.TileContext(nc) as tc:
        with (
            tc.tile_pool(name="const", bufs=1) as cpool,
            tc.tile_pool(name="state", bufs=1) as spool,
            tc.tile_pool(name="work", bufs=3) as wpool,
            tc.tile_pool(name="ps_z", bufs=1, space="PSUM") as ps_z,
            tc.tile_pool(name="ps_h", bufs=1, space="PSUM") as ps_h,
            tc.tile_pool(name="ps_y", bufs=1, space="PSUM") as ps_y,
            tc.tile_pool(name="ps_cs", bufs=1, space="PSUM") as ps_cs,
            tc.tile_pool(name="ps_wf", bufs=1, space="PSUM") as ps_wf,
            tc.tile_pool(name="ps_w", bufs=1, space="PSUM") as ps_w,
        ):
            # ---------------- constants into SBUF ----------------
            sb_trans = cpool.tile([U, BL, N], F32)
            nc.sync.dma_start(out=sb_trans, in_=trans_u[:, :].rearrange("u (b n) -> u b n", b=BL))
            sb_wx = cpool.tile([IN_P, G4], F32)
            nc.sync.dma_start(out=sb_wx, in_=wx[:, :])
            sb_wh = cpool.tile([128, 4, G4], F32)
            nc.sync.dma_start(out=sb_wh, in_=wh[:, :].rearrange("(k p) g -> p k g", p=128))
            sb_wd = cpool.tile([128, 4, 3 * M], F32)
            nc.sync.dma_start(out=sb_wd, in_=wd[:, :].rearrange("(k p) g -> p k g", p=128))
            sb_bd = cpool.tile([BL, 3 * M], F32)
            nc.sync.dma_start(out=sb_bd, in_=bd2[:, :])
            sb_masks = cpool.tile([BL, 3 * BL * M], F32)
            nc.sync.dma_start(out=sb_masks, in_=masks[:, :])
            sb_ones = cpool.tile([BL, 128], F32)
            nc.sync.dma_start(out=sb_ones, in_=ones16[:, :])
            sb_id16 = cpool.tile([BL, BL], F32)
            nc.sync.dma_start(out=sb_id16, in_=id16[:, :])
            sb_id128 = cpool.tile([128, 128], F32)
            nc.sync.dma_start(out=sb_id128, in_=id128[:, :])
            sb_enums = cpool.tile([128, 2], F32)
            nc.sync.dma_start(out=sb_enums, in_=enums[:, :])

            # ---------------- persistent state ----------------
            h_T = spool.tile([128, 4, BL], F32)     # transposed hidden state
            X = spool.tile([128, H], F32)           # rows 0:16 tanh_g, 32:48 c, 96:112 tanh_c
            # inpx: per-step z lhsT columns. rows 0:80 = w_{t-1} (written by
            # step t-1), rows 96:99 = x_t (preloaded for all t), rest zero.
            inpx = spool.tile([128, T + 1, BL], F32)
            kap = spool.tile([BL, M], F32)          # kappa [16,10]

            nc.vector.memset(h_T, 0.0)
            nc.vector.memset(X, 0.0)
            nc.vector.memset(inpx, 0.0)
            nc.vector.memset(kap, 0.0)
            nc.sync.dma_start(
                out=inpx[96:99, 0:T, :],
                in_=strokes_t[:, :].rearrange("c (t b) -> c t b", b=BL),
            )

            enum1 = sb_enums[:, 0:1]
            enum2 = sb_enums[:, 1:2]

            for t in range(T):
                # ---- z matmuls: col-tiled, 5 K-chunks x 4 gate banks ----
                zb = ps_z.tile([128, 512], F32)
                for k in range(5):
                    if k < 4:
                        lhsT = h_T[:, k, :]
                        rhs_full = sb_wh[:, k, :]
                    else:
                        lhsT = inpx[:, t, :]
                        rhs_full = sb_wx[:, :]
                    for g in range(4):
                        nc.tensor.matmul(
                            zb[32 * g : 32 * g + BL, :],
                            bc(lhsT, zdt),
                            bc(rhs_full[:, ds(512 * g, 512)], zdt),
                            start=(k == 0),
                            stop=(k == 4),
                            tile_position=(0, 32 * g),
                            skip_group_check=True,
                        )

                # ---- LSTM pointwise (batch-major slabs) ----
                # NOTE: walrus requires equal base partitions for both inputs of
                # 2-input vector ops, so operands are co-located by base row.
                S = wpool.tile([128, H], F32, tag="S")
                # sigmoid over all partitions: rows 0:16 = sig_i, 32:48 = sig_f, 96:112 = sig_o
                nc.scalar.activation(S, zb, AF.Sigmoid)
                # tanh(g) -> X rows 0:16   (c state lives at X rows 32:48)
                nc.scalar.activation(X[0:BL, :], zb[64 : 64 + BL, :], AF.Tanh)
                # ig = sig_i * tanh_g (gpsimd, off the DVE critical path), at rows 32:48
                ig_t = wpool.tile([48, H], F32, tag="ig")
                nc.gpsimd.tensor_mul(ig_t[32 : 32 + BL, :], S[0:BL, :], X[0:BL, :])
                # fc = sig_f * c, at rows 32:48
                fc_t = wpool.tile([48, H], F32, tag="fc")
                nc.vector.tensor_mul(
                    fc_t[32 : 32 + BL, :], S[32 : 32 + BL, :], X[32 : 32 + BL, :]
                )
                # c_new -> X rows 32:48
                nc.vector.tensor_add(
                    X[32 : 32 + BL, :], fc_t[32 : 32 + BL, :], ig_t[32 : 32 + BL, :]
                )
                # tanh(c_new) -> X rows 96:112
                nc.scalar.activation(X[96 : 96 + BL, :], X[32 : 32 + BL, :], AF.Tanh)
                h_bm = wpool.tile([BL, H], F32, tag="hbm")
                nc.vector.tensor_mul(h_bm, S[96 : 96 + BL, :], X[96 : 96 + BL, :])

                nc.sync.dma_start(out=o_h[t], in_=h_bm)

                # ---- h_bm -> h_T via 4 PE transposes ----
                h_ps = ps_h.tile([128, 4, BL], F32)
                for k in range(4):
                    nc.tensor.transpose(h_ps[:, k, :], h_bm[:, ds(128 * k, 128)], sb_id16)
                nc.vector.tensor_copy(h_T, h_ps)

                # ---- y = h @ Wd (batch-major out), + bd ----
                y_ps = ps_y.tile([BL, 3 * M], F32)
                for k in range(4):
                    nc.tensor.matmul(
                        y_ps,
                        bc(h_T[:, k, :], ydt),
                        bc(sb_wd[:, k, :], ydt),
                        start=(k == 0),
                        stop=(k == 3),
                    )
                y2 = wpool.tile([BL, 3 * M], F32, tag="y2")
                nc.vector.tensor_add(y2, y_ps, sb_bd)

                # ---- exp for beta, kappa_inc; kappa update ----
                exbk = wpool.tile([BL, 2 * M], F32, tag="exbk")
                nc.scalar.activation(exbk, y2[:, M : 3 * M], AF.Exp)
                nc.vector.tensor_add(kap, kap, exbk[:, M : 2 * M])

                # ---- c0 = d_alpha - beta*kappa^2, c1src = beta*kappa, c2src = beta ----
                bk = wpool.tile([BL, M], F32, tag="bk")
                nc.vector.tensor_mul(bk, exbk[:, 0:M], kap)
                t1 = wpool.tile([BL, M], F32, tag="t1")
                nc.vector.tensor_mul(t1, bk, kap)
                c0b = wpool.tile([BL, M], F32, tag="c0b")
                nc.vector.tensor_sub(c0b, y2[:, 0:M], t1)

                # ---- spread to (b,m) free cols via masked broadcast ----
                Cs = wpool.tile([BL, 3 * BL * M], F32, tag="Cs")
                BM = BL * M  # 160
                nc.gpsimd.tensor_mul(
                    Cs[:, 0:BM],
                    sb_masks[:, 0:BM],
                    c0b[:, None, :].to_broadcast((BL, BL, M)),
                )
                nc.gpsimd.tensor_mul(
                    Cs[:, BM : 2 * BM],
                    sb_masks[:, BM : 2 * BM],
                    bk[:, None, :].to_broadcast((BL, BL, M)),
                )
                nc.gpsimd.tensor_mul(
                    Cs[:, 2 * BM : 3 * BM],
                    sb_masks[:, 2 * BM : 3 * BM],
                    exbk[:, None, 0:M].to_broadcast((BL, BL, M)),
                )
                cs_bc = ps_cs.tile([128, 3 * BM], F32)
                nc.tensor.matmul(cs_bc, sb_ones, Cs, start=True, stop=True)

                # ---- phi exponent polynomial + exp, reduce over m ----
                # Horner: s = (c2*u + c1)*u + c0, max one PSUM input per op
                tmp1 = wpool.tile([128, BM], F32, tag="tmp1")
                nc.vector.tensor_scalar(
                    tmp1, cs_bc[:, 2 * BM : 3 * BM], enum1, None, op0=ALU.mult
                )
                tmp2 = wpool.tile([128, BM], F32, tag="tmp2")
                nc.vector.tensor_add(tmp2, tmp1, cs_bc[:, BM : 2 * BM])
                s_sb = wpool.tile([128, BM], F32, tag="s_sb")
                nc.vector.scalar_tensor_tensor(
                    s_sb, tmp2, enum1, cs_bc[:, 0:BM], op0=ALU.mult, op1=ALU.add,
                )
                phi_T = wpool.tile([128, BL, M], F32, tag="phi")
                nc.scalar.activation(phi_T, s_sb.rearrange("p (b m) -> p b m", b=BL), AF.Exp)
                wfull_T = wpool.tile([128, BL], F32, tag="wfT")
                nc.vector.tensor_reduce(wfull_T, phi_T, axis=mybir.AxisListType.X, op=ALU.add)

                # ---- wfull batch-major (u 0:128 via PE transpose; u=128 tail) ----
                wf_ps = ps_wf.tile([BL, 128], F32)
                nc.tensor.transpose(wf_ps, wfull_T, sb_id128)
                wf_sb = wpool.tile([BL, U + 1], F32, tag="wfsb")
                nc.scalar.copy(wf_sb[:, 0:128], wf_ps)
                # tail: s128 = c0 + 256*bk - 16384*beta  (u=128)
                v1t = wpool.tile([BL, M], F32, tag="v1t")
                nc.vector.scalar_tensor_tensor(
                    v1t, bk, float(2 * U), c0b, op0=ALU.mult, op1=ALU.add
                )
                v2t = wpool.tile([BL, M], F32, tag="v2t")
                nc.vector.scalar_tensor_tensor(
                    v2t, exbk[:, 0:M], float(-U * U), v1t, op0=ALU.mult, op1=ALU.add
                )
                e128 = wpool.tile([BL, M], F32, tag="e128")
                nc.scalar.activation(e128, v2t, AF.Exp)
                nc.vector.tensor_reduce(
                    wf_sb[:, 128:129], e128, axis=mybir.AxisListType.X, op=ALU.add
                )

                # ---- argmax ----
                mx8 = wpool.tile([BL, 8], F32, tag="mx8")
                nc.vector.max(mx8, wf_sb)
                idx8 = wpool.tile([BL, 8], U32, tag="idx8")
                nc.vector.max_index(idx8, mx8, wf_sb)
                nc.sync.dma_start(out=o_idx[t], in_=idx8[:, 0:1])

                # ---- w = wfull[:, :U] @ trans[b]  (16 small matmuls) ----
                w_ps = ps_w.tile([N, BL], F32)
                for b in range(BL):
                    nc.tensor.matmul(
                        w_ps[:, b : b + 1],
                        sb_trans[:, b, :],
                        wfull_T[:, b : b + 1],
                        start=True,
                        stop=True,
                        skip_group_check=True,
                    )
                nc.vector.tensor_copy(inpx[0:N, t + 1, :], w_ps)
                w_out = wpool.tile([N, BL], F32, tag="w_out")
                nc.scalar.copy(w_out, w_ps)
                nc.sync.dma_start(out=o_w[t], in_=w_out)

    nc.compile()
    return nc


def make_host_constants():
    """Shared constant arrays fed to every core."""
    f32 = np.float32
    bd = np.concatenate([np.zeros(2 * M, f32), -4.0 * np.ones(M, f32)])
    bd2 = np.tile(bd[None, :], (BL, 1)).astype(f32)
    # masks [16, 3, 16, 10]: delta(b'==b) * {1, 2, -1}
    eye = np.eye(BL, dtype=f32)
    mk = np.zeros((BL, 3, BL, M), f32)
    mk[:, 0] = eye[:, :, None] * 1.0
    mk[:, 1] = eye[:, :, None] * 2.0
    mk[:, 2] = eye[:, :, None] * -1.0
    masks = mk.reshape(BL, 3 * BL * M)
    ones16 = np.ones((BL, 128), f32)
    id16 = np.eye(BL, dtype=f32)
    id128 = np.eye(128, dtype=f32)
    uu = np.arange(128, dtype=f32)
    enums = np.stack([uu, uu * uu], axis=1)  # [128, 2]
    return dict(bd2=bd2, masks=masks, ones16=ones16, id16=id16, id128=id128, enums=enums)


def make_core_inputs(strokes, transcriptions, Wx, Wh, Wd, T, consts):
    """Per-core input maps. strokes [B,T,3], transcriptions [B,U,N]."""
    f32 = np.float32
    wxp = np.zeros((IN_P, G4), f32)
    wxp[0:N] = Wx[3 : 3 + N]
    wxp[96:99] = Wx[0:3]
    maps = []
    for c in range(NCORES):
        sl = slice(c * BL, (c + 1) * BL)
        st = strokes[sl, :T, :]  # [16, T, 3]
        strokes_t = np.ascontiguousarray(st.transpose(2, 1, 0)).reshape(3, T * BL)
        tr = transcriptions[sl]  # [16, 128, 80]
        trans_u = np.ascontiguousarray(tr.transpose(1, 0, 2)).reshape(U, BL * N)
        m = dict(
            strokes_t=strokes_t.astype(f32),
            trans_u=trans_u.astype(f32),
            wx=wxp,
            wh=Wh.astype(f32),
            wd=Wd.astype(f32),
            **{k: v.copy() for k, v in consts.items()},
        )
        maps.append(m)
    return maps


def assemble_output(results, T):
    """results: list of per-core dicts -> full [B, T, H+N+1] f32."""
    outs = []
    for c in range(NCORES):
        r = results[c]
        oh = r["o_h"]          # [T, 16, 512]
        ow = r["o_w"]          # [T, 80, 16]
        oi = r["o_idx"]        # [T, 16] uint32
        full = np.empty((BL, T, H + N + 1), np.float32)
        full[:, :, :H] = oh.transpose(1, 0, 2)
        full[:, :, H : H + N] = ow.transpose(2, 0, 1)
        full[:, :, H + N] = oi.astype(np.float32).T
        outs.append(full)
    return np.concatenate(outs, axis=0)


_CACHE = {}


def run(strokes, transcriptions, Wx, Wh, b, Wd, bd, T=T_FULL, trace=False,
        z_dtype="f32r", y_dtype="f32r"):
    key = (T, z_dtype, y_dtype)
    if key not in _CACHE:
        _CACHE[key] = build_program(T, z_dtype=z_dtype, y_dtype=y_dtype)
    nc = _CACHE[key]
    consts = make_host_constants()
    # general-case: honor the actual b/bd inputs (b is zeros, bd -4 block by spec,
    # but bd2 comes from the real bd; b folded into nothing since it is zeros —
    # assert to be safe)
    assert np.abs(b).max() == 0.0, "nonzero LSTM bias not supported"
    consts["bd2"] = np.tile(np.asarray(bd, np.float32)[None, :], (BL, 1))
    in_maps = make_core_inputs(strokes, transcriptions, Wx, Wh, Wd, T, consts)
    res = run_bass_kernel_spmd(nc, in_maps, list(range(NCORES)), trace=trace)
    out = assemble_output(res.results, T)
    return out, res


def kernel(**inputs):
    strokes = np.asarray(inputs["strokes"], np.float32)
    transcriptions = np.asarray(inputs["transcriptions"], np.float32)
    Wx = np.asarray(inputs["Wx"], np.float32)
    Wh = np.asarray(inputs["Wh"], np.float32)
    b = np.asarray(inputs["b"], np.float32)
    Wd = np.asarray(inputs["Wd"], np.float32)
    bd = np.asarray(inputs["bd"], np.float32)
    out, _ = run(strokes, transcriptions, Wx, Wh, b, Wd, bd, T=strokes.shape[1])
    return out
